# revision 1
# baseline (speedup 1.0000x reference)
"""Trainium2 Bass kernel: 2-layer LSTM seq2seq + attention beam-search decoder
(V=32000, H=512, K=3 beams, S=64, T=50) on 8 NeuronCores.

v2 optimizations over the f32r baseline:
- All heavy weight-stationary matmuls use an exact-enough 2-term bf16 split
  (w = b0+b1, x = c0+c1; w@x ~ b0c0 + b1c0 + b0c1, fp32 PSUM accumulation).
  bf16 LDWEIGHTS runs ~6x faster than f32r (FWL), so 3 passes beat 1 by ~2.1x.
  CPU-simulated end to end: trajectory is token-exact vs the fp32 reference.
- Whh0@h1 / Whh1@h2 (M1/M2) are computed right after the logits matmuls so
  they overlap the AllGather+merge window; the beam reorder is applied
  post-merge as a column select (exact by linearity of the matmul).
- LSTM gates reordered host-side to [i,f,o,g] so each cell needs only two
  activation calls (sigmoid over 3 gate blocks, tanh over 1).
"""
import sys
sys.path.insert(0, '/opt/trn_rl_repo')
import contextlib
import numpy as np
import ml_dtypes
import concourse.bass as bass
import concourse.bacc as bacc
import concourse.mybir as mybir
import concourse.tile as tile
from concourse import bass_utils

f32 = mybir.dt.float32
f32r = mybir.dt.float32r
bf16 = mybir.dt.bfloat16
i32 = mybir.dt.int32
u32 = mybir.dt.uint32

V, H, L, S, K, TMAX, BOS = 32000, 512, 2, 64, 3, 50, 1
NC = 8
VSH = V // NC
BS = 4                  # beam slots (3 real + 1 dummy)
KC = H // 128           # 4
MC = (4 * H) // 128     # 16
NEG = -1.0e30

AF = mybir.ActivationFunctionType
ALU = mybir.AluOpType
AX = mybir.AxisListType

# gate permutation: [i, f, g, o] -> [i, f, o, g] (by 128-row m-tiles)
GPERM = [0, 1, 2, 3, 4, 5, 6, 7, 12, 13, 14, 15, 8, 9, 10, 11]


def _gate_perm_rows(W):
    """Permute rows of [4H, ...] weight from (i,f,g,o) to (i,f,o,g)."""
    W4 = W.reshape(MC, 128, -1)
    return np.ascontiguousarray(W4[GPERM].reshape(W.shape))


def _tiles_lhsT(Wt):
    """Wt [Kdim, Mdim] -> [kc, mc, 128, 128], tile[k][m] = Wt[k*128:.., m*128:..]."""
    Kd, Md = Wt.shape
    return np.ascontiguousarray(
        Wt.reshape(Kd // 128, 128, Md // 128, 128).transpose(0, 2, 1, 3))


def _bsplit(x):
    b0 = np.asarray(x).astype(ml_dtypes.bfloat16)
    b1 = (np.asarray(x, np.float32) - b0.astype(np.float32)).astype(ml_dtypes.bfloat16)
    return b0, b1


def _tiles_lhsT_split(Wt):
    b0, b1 = _bsplit(Wt)
    return _tiles_lhsT(b0), _tiles_lhsT(b1)


def host_prep(inputs):
    g = {k: np.asarray(v) for k, v in inputs.items()}
    emb_enc = g['emb_enc'].astype(np.float32)
    emb_dec = g['emb_dec'].astype(np.float32)
    Wih_enc = np.stack([_gate_perm_rows(g['Wih_enc'].astype(np.float32)[l]) for l in range(L)])
    Whh_enc = np.stack([_gate_perm_rows(g['Whh_enc'].astype(np.float32)[l]) for l in range(L)])
    Wih_dec = np.stack([_gate_perm_rows(g['Wih_dec'].astype(np.float32)[l]) for l in range(L)])
    Whh_dec = np.stack([_gate_perm_rows(g['Whh_dec'].astype(np.float32)[l]) for l in range(L)])
    W_c = g['W_c'].astype(np.float32)
    W_out = g['W_out'].astype(np.float32)
    seq = np.asarray(g['input_seq']).reshape(-1).astype(np.int64)

    com = {}
    com['EM4'] = np.ascontiguousarray(emb_enc.reshape(V * KC, 128))
    offs = np.empty((2, 128, 1), np.int32)
    for q in range(2 * 128):
        k, t = q // S, q % S
        offs[q // 128, q % 128, 0] = int(seq[t]) * KC + k
    com['ENCOFF'] = offs
    for l in range(L):
        a, b = _tiles_lhsT_split(Wih_enc[l].T)
        com[f'WihT_enc{l}a'], com[f'WihT_enc{l}b'] = a, b
        a, b = _tiles_lhsT_split(Whh_enc[l].T)
        com[f'WhhT_enc{l}a'], com[f'WhhT_enc{l}b'] = a, b
    EW = (emb_dec @ Wih_dec[0].T).astype(np.float32)
    com['EW4'] = np.ascontiguousarray(EW.reshape(V * MC, 128))
    a, b = _tiles_lhsT_split(Wih_dec[1].T)
    com['WihT_dec1a'], com['WihT_dec1b'] = a, b
    for l in range(L):
        a, b = _tiles_lhsT_split(Whh_dec[l].T)
        com[f'WhhT_dec{l}a'], com[f'WhhT_dec{l}b'] = a, b
    a, b = _tiles_lhsT_split(W_c[:, :H].T)
    com['WcxTa'], com['WcxTb'] = a, b
    wcc0, wcc1 = _bsplit(W_c[:, H:].T)
    com['WccTa'] = np.ascontiguousarray(np.asarray(wcc0).reshape(KC, 128, H))
    com['WccTb'] = np.ascontiguousarray(np.asarray(wcc1).reshape(KC, 128, H))
    com['EYE'] = np.eye(128, dtype=np.float32)
    com['ONES1x128'] = np.ones((1, 128), np.float32)
    sb = np.zeros((BS, MC * BS), np.float32)
    mo = np.zeros((MC * BS, 1), np.float32)
    for q in range(MC * BS):
        m, b2 = q // BS, q % BS
        sb[b2, q] = float(MC)            # off = MC*tok_b + m
        mo[q, 0] = float(m)
    com['SelB64T'] = sb
    com['Moff64'] = mo
    mask = np.zeros((128, 128), np.float32)
    for m in range(32):
        for b2 in range(BS):
            col = m * BS + b2
            if b2 == 3:
                mask[:, col] = NEG
            elif m == 31:
                mask[32:, col] = NEG
    com['LMASK'] = mask
    com['ONESCOL'] = np.ones((128, 1), np.float32)
    com['IOTA9'] = np.tile(np.arange(9, dtype=np.float32)[None, :], (BS, 1))
    com['ORIG9'] = np.tile((np.arange(9, dtype=np.float32) // 3)[None, :], (BS, 1))
    com['IOTA768'] = np.tile(np.arange(NC * 32 * 3, dtype=np.float32)[None, :], (BS, 1))
    aC = np.zeros((128, 16), np.float32)
    for a2 in range(4):
        aC[:, a2 * 4:(a2 + 1) * 4] = float(a2)
    com['AC16'] = aC
    aC3 = np.zeros((128, 48), np.float32)
    for a2 in range(4):
        aC3[:, a2 * 4:(a2 + 1) * 4] = float(a2)
        aC3[:, 16 + a2 * 4:16 + (a2 + 1) * 4] = float(a2)
        aC3[:, 32 + a2 * 4:32 + (a2 + 1) * 4] = float(a2)
    com['IOTAP4'] = np.arange(4, dtype=np.float32).reshape(4, 1)
    com['INIT_SC'] = np.array([[0.0], [-1e9], [-2e9], [NEG]], np.float32)
    com['INIT_TOK'] = np.array([[float(BOS)]] * 3 + [[0.0]], np.float32)
    com['NEG8'] = np.full((1, 8), NEG, np.float32)
    fm = np.zeros((4, 18), np.float32)
    for b2 in range(3):
        fm[b2, b2 * 6:(b2 + 1) * 6] = 1.0
    com['FMASK18'] = fm

    per_core = []
    W_outT = np.ascontiguousarray(W_out.T)      # [512, 32000]
    for c in range(NC):
        d = {}
        sh = np.zeros((H, 4096), np.float32)
        sh[:, :VSH] = W_outT[:, c * VSH:(c + 1) * VSH]
        a, b = _tiles_lhsT_split(sh)
        d['WoTa'], d['WoTb'] = a, b
        rb = np.zeros((128, 1), np.float32)     # vocab-id base: c*4000 + m*128
        for q in range(128):
            rb[q, 0] = float(c * VSH + (q // BS) * 128)
        d['ROWBASE'] = rb
        per_core.append(d)
    return com, per_core


def build(T_steps=TMAX):
    nc = bacc.Bacc("TRN2", target_bir_lowering=False, debug=False, num_devices=NC)

    def din(name, shape, dtype=f32):
        return nc.dram_tensor(name, list(shape), dtype, kind="ExternalInput")

    EM4 = din('EM4', [V * KC, 128])
    ENCOFF = din('ENCOFF', [2, 128, 1], i32)
    WihT_enc = [(din(f'WihT_enc{l}a', [KC, MC, 128, 128], bf16),
                 din(f'WihT_enc{l}b', [KC, MC, 128, 128], bf16)) for l in range(L)]
    WhhT_enc = [(din(f'WhhT_enc{l}a', [KC, MC, 128, 128], bf16),
                 din(f'WhhT_enc{l}b', [KC, MC, 128, 128], bf16)) for l in range(L)]
    EW4 = din('EW4', [V * MC, 128])
    WihT_dec1 = (din('WihT_dec1a', [KC, MC, 128, 128], bf16),
                 din('WihT_dec1b', [KC, MC, 128, 128], bf16))
    WhhT_dec = [(din(f'WhhT_dec{l}a', [KC, MC, 128, 128], bf16),
                 din(f'WhhT_dec{l}b', [KC, MC, 128, 128], bf16)) for l in range(L)]
    WcxT = (din('WcxTa', [KC, KC, 128, 128], bf16), din('WcxTb', [KC, KC, 128, 128], bf16))
    WccT = (din('WccTa', [KC, 128, H], bf16), din('WccTb', [KC, 128, H], bf16))
    WoT_d = (din('WoTa', [KC, 32, 128, 128], bf16), din('WoTb', [KC, 32, 128, 128], bf16))
    EYE = din('EYE', [128, 128])
    ONES1x128 = din('ONES1x128', [1, 128])
    SelB64T = din('SelB64T', [BS, MC * BS])
    Moff64 = din('Moff64', [MC * BS, 1])
    LMASK = din('LMASK', [128, 128])
    ONESCOL = din('ONESCOL', [128, 1])
    IOTA9 = din('IOTA9', [BS, 9])
    ORIG9 = din('ORIG9', [BS, 9])
    IOTA768 = din('IOTA768', [BS, NC * 32 * 3])
    AC16 = din('AC16', [128, 16])
    IOTAP4 = din('IOTAP4', [4, 1])
    INIT_SC = din('INIT_SC', [4, 1])
    INIT_TOK = din('INIT_TOK', [4, 1])
    NEG8 = din('NEG8', [1, 8])
    FMASK18 = din('FMASK18', [4, 18])
    ROWBASE = din('ROWBASE', [128, 1])

    OUT = nc.dram_tensor('out_tokens', [1, TMAX], i32, kind="ExternalOutput")

    with tile.TileContext(nc) as tc:
        with contextlib.ExitStack() as octx:
            spool = octx.enter_context(tc.tile_pool(name="state", bufs=1))
            dpool = octx.enter_context(tc.tile_pool(name="dram", bufs=1, space="DRAM"))

            def loadc(t_dram, shape, dtype=f32, tag=None, pool=None):
                tl = (pool or spool).tile(list(shape), dtype, tag=tag)
                src = t_dram[:] if dtype in (f32, i32, u32, bf16) else t_dram[:].bitcast(dtype)
                nc.sync.dma_start(tl[:], src)
                return tl

            eye = loadc(EYE, [128, 128], tag='eye')
            ones1 = loadc(ONES1x128, [1, 128], tag='ones1')
            selb = loadc(SelB64T, [BS, MC * BS], tag='selb')
            moff = loadc(Moff64, [MC * BS, 1], tag='moff')
            lmask = loadc(LMASK, [128, 128], tag='lmask')
            onescol = loadc(ONESCOL, [128, 1], tag='onescol')
            iota9 = loadc(IOTA9, [BS, 9], tag='iota9')
            orig9 = loadc(ORIG9, [BS, 9], tag='orig9')
            iota768 = loadc(IOTA768, [BS, NC * 32 * 3], tag='iota768')
            ac16 = loadc(AC16, [128, 16], tag='ac16')
            iotap4 = loadc(IOTAP4, [4, 1], tag='iotap4')
            neg8 = loadc(NEG8, [1, 8], tag='neg8')
            fmask18 = loadc(FMASK18, [4, 18], tag='fmask18')
            rowbase = loadc(ROWBASE, [128, 1], tag='rowbase')

            def wtiles2(t_pair, kc, mc, tag, pool):
                out = []
                for s, t_dram in enumerate(t_pair):
                    tl = pool.tile([128, kc * mc * 128], bf16, tag=f'{tag}_{s}')
                    nc.sync.dma_start(
                        tl[:].rearrange("p (k m q) -> p k m q", k=kc, m=mc, q=128),
                        t_dram.rearrange("k m p q -> p k m q"))
                    out.append(tl)
                return out

            def tof(tl, mc_total, k, m):
                off = (k * mc_total + m) * 128
                return tl[:, off:off + 128]

            def mm3(psl, wpair, mc_total, k, m, rhs_pair, sl, start, stop):
                """3-pass split matmul accumulate: b0c0 + b1c0 + b0c1."""
                w0, w1 = wpair
                r0 = rhs_pair[0][:, sl[0]:sl[1]]
                r1 = rhs_pair[1][:, sl[0]:sl[1]]
                nc.tensor.matmul(psl, tof(w0, mc_total, k, m), r0,
                                 start=start, stop=False)
                nc.tensor.matmul(psl, tof(w1, mc_total, k, m), r0,
                                 start=False, stop=False)
                nc.tensor.matmul(psl, tof(w0, mc_total, k, m), r1,
                                 start=False, stop=stop)

            def split_tiles(pool, src_ap, ncols, tag, scratch_tag=None):
                """Make bf16 pair (c0, c1) from f32 source AP [128, ncols]."""
                s0 = pool.tile([128, ncols], bf16, tag=f'{tag}0')
                s1 = pool.tile([128, ncols], bf16, tag=f'{tag}1')
                tmp = pool.tile([128, ncols], f32, tag=f'{tag}t')
                nc.vector.tensor_copy(s0[:], src_ap)
                nc.vector.tensor_copy(tmp[:], s0[:])
                nc.vector.tensor_tensor(tmp[:], src_ap, tmp[:], op=ALU.subtract)
                nc.vector.tensor_copy(s1[:], tmp[:])
                return s0, s1

            enc_outT = spool.tile([128, KC * S], f32r, tag='encoutT')
            EOC = spool.tile([64, H], f32r, tag='EOC')
            hc = spool.tile([128, 64], f32, tag='hc')       # h1|c1|h2|c2 (16 cols each)
            hh = spool.tile([128, 32], f32r, tag='hh')      # h1|h2 f32r
            btx = spool.tile([4, 72], f32, tag='btx')       # bt 0..63, col 64 = tokens
            scores = spool.tile([4, 1], f32, tag='scores')
            SM = spool.tile([128, 192], f32, tag='SM')      # hcM: M1s | M2s at 64:192

            # =========================== ENCODER ===========================
            with contextlib.ExitStack() as ectx:
                ewpool = ectx.enter_context(tc.tile_pool(name="encw", bufs=1))
                epool = ectx.enter_context(tc.tile_pool(name="encs", bufs=2))
                eppool = ectx.enter_context(tc.tile_pool(name="encp", bufs=2, space="PSUM"))

                whh_e = [wtiles2(WhhT_enc[l], KC, MC, f'whhe{l}', ewpool) for l in range(L)]
                wih_e = [wtiles2(WihT_enc[l], KC, MC, f'wihe{l}', ewpool) for l in range(L)]
                wcc = []
                for s2 in range(2):
                    t2 = ewpool.tile([128, KC * H], bf16, tag=f'wcc{s2}')
                    nc.sync.dma_start(t2[:].rearrange("p (k h) -> p k h", k=KC, h=H),
                                      WccT[s2].rearrange("k p h -> p k h"))
                    wcc.append(t2)

                xTenc = epool.tile([128, KC * S], f32, tag='xTenc')
                for half in range(2):
                    offt = epool.tile([128, 1], i32, tag='encoff')
                    nc.sync.dma_start(offt[:], ENCOFF[half])
                    gat = epool.tile([128, 128], f32, tag='encgat')
                    nc.gpsimd.indirect_dma_start(
                        out=gat[:], out_offset=None, in_=EM4[:],
                        in_offset=bass.IndirectOffsetOnAxis(ap=offt[:, :1], axis=0))
                    pt = eppool.tile([128, 128], f32, tag='encT')
                    nc.tensor.transpose(pt[:], gat[:], eye[:])
                    nc.vector.tensor_copy(xTenc[:, half * 128:(half + 1) * 128], pt[:])

                def batched_zih(wih, xs, zname):
                    zT = epool.tile([128, MC * S], f32, tag=zname)
                    for m in range(MC):
                        pt = eppool.tile([128, S], f32, tag='zihp')
                        for k in range(KC):
                            mm3(pt[:], wih, MC, k, m, xs, (k * S, (k + 1) * S),
                                start=(k == 0), stop=(k == KC - 1))
                        nc.vector.tensor_copy(zT[:, m * S:(m + 1) * S], pt[:])
                    return zT

                def enc_layer(zihT, whh, out_xT):
                    hT = epool.tile([128, KC * 2], f32, tag='ehT')
                    cT = epool.tile([128, 4], f32, tag='ecT')
                    nc.vector.memset(hT[:], 0.0)
                    nc.vector.memset(cT[:], 0.0)
                    # zih viewed as [p, gate(4), hblock(4), t]
                    zih4 = zihT[:].rearrange("p (g q s) -> p g q s", g=4, q=4, s=S)
                    out3 = out_xT[:].rearrange("p (k s) -> p k s", k=KC, s=S)
                    hT3 = hT[:].rearrange("p (k o) -> p k o", k=KC, o=2)
                    for t in range(S):
                        hs = split_tiles(epool, hT[:], KC * 2, 'ehs')
                        # process h-dim quarters so gate math overlaps matmuls
                        for q in range(KC):
                            zp = eppool.tile([128, 8], f32, tag='ezp')
                            for gi in range(4):
                                m = gi * 4 + q
                                for k in range(KC):
                                    mm3(zp[:, gi * 2:(gi + 1) * 2], whh, MC, k, m,
                                        hs, (k * 2, (k + 1) * 2),
                                        start=(gi == 0 and k == 0),
                                        stop=(gi == 3 and k == KC - 1))
                            zg = epool.tile([128, 4], f32, tag=f'ezg{q % 2}')
                            nc.vector.tensor_tensor(
                                zg[:].rearrange("p (g o) -> p g o", o=1),
                                zp[:].rearrange("p (g o) -> p g o", o=2)[:, :, 0:1],
                                zih4[:, :, q, t:t + 1], op=ALU.add)
                            # zg cols: [i, f, o, g] for this h-block
                            sio = epool.tile([128, 3], f32, tag=f'esio{q % 2}')
                            tg = epool.tile([128, 1], f32, tag=f'etg{q % 2}')
                            nc.scalar.activation(sio[:], zg[:, 0:3], AF.Sigmoid)
                            nc.scalar.activation(tg[:], zg[:, 3:4], AF.Tanh)
                            cq = cT[:, q:q + 1]
                            nc.vector.tensor_tensor(cq, sio[:, 1:2], cq, op=ALU.mult)
                            si2 = epool.tile([128, 1], f32, tag=f'esi2{q % 2}')
                            nc.vector.tensor_tensor(si2[:], sio[:, 0:1], tg[:], op=ALU.mult)
                            nc.vector.tensor_tensor(cq, cq, si2[:], op=ALU.add)
                            tc_ = epool.tile([128, 1], f32, tag=f'etc{q % 2}')
                            nc.scalar.activation(tc_[:], cq, AF.Tanh)
                            hn = epool.tile([128, 1], f32, tag=f'ehn{q % 2}')
                            nc.vector.tensor_tensor(hn[:], sio[:, 2:3], tc_[:], op=ALU.mult)
                            nc.vector.tensor_copy(hT3[:, q, 0:1], hn[:])
                            nc.vector.tensor_copy(out3[:, q, t:t + 1], hn[:])
                    return hT, cT

                xs0 = split_tiles(epool, xTenc[:], KC * S, 'xs0')
                zih0T = batched_zih(wih_e[0], xs0, 'zih0T')
                x1T = epool.tile([128, KC * S], f32, tag='x1T')
                h0, c0 = enc_layer(zih0T, whh_e[0], x1T)
                xs1 = split_tiles(epool, x1T[:], KC * S, 'xs1')
                zih1T = batched_zih(wih_e[1], xs1, 'zih1T')
                x2T = epool.tile([128, KC * S], f32, tag='x2T')
                h1, c1 = enc_layer(zih1T, whh_e[1], x2T)
                nc.vector.tensor_copy(enc_outT[:], x2T[:])

                es = split_tiles(epool, x2T[:], KC * S, 'encsp')
                peoc = eppool.tile([64, H], f32, tag='peoc')
                for k in range(KC):
                    nc.tensor.matmul(
                        peoc[:], es[0][:, k * S:(k + 1) * S],
                        wcc[0][:, k * H:(k + 1) * H],
                        start=(k == 0), stop=False)
                    nc.tensor.matmul(
                        peoc[:], es[1][:, k * S:(k + 1) * S],
                        wcc[0][:, k * H:(k + 1) * H],
                        start=False, stop=False)
                    nc.tensor.matmul(
                        peoc[:], es[0][:, k * S:(k + 1) * S],
                        wcc[1][:, k * H:(k + 1) * H],
                        start=False, stop=(k == KC - 1))
                nc.vector.tensor_copy(EOC[:], peoc[:])

                for g, st in ((0, h0), (2, h1)):
                    nc.vector.tensor_copy(
                        hc[:, g * 16:(g + 1) * 16].rearrange("p (k b) -> p k b", k=4, b=4),
                        st[:].rearrange("p (k o) -> p k o", k=KC, o=2)[:, :, 0:1]
                        .to_broadcast([128, 4, 4]))
                for g, st in ((1, c0), (3, c1)):
                    nc.vector.tensor_copy(
                        hc[:, g * 16:(g + 1) * 16].rearrange("p (k b) -> p k b", k=4, b=4),
                        st[:].rearrange("p (k o) -> p k o", k=KC, o=1)
                        .to_broadcast([128, 4, 4]))
                nc.vector.tensor_copy(hh[:, 0:16], hc[:, 0:16])
                nc.vector.tensor_copy(hh[:, 16:32], hc[:, 32:48])
                nc.vector.tensor_copy(SM[:, 0:64], hc[:])

            # =========================== DECODER ===========================
            with contextlib.ExitStack() as dctx:
                dwpool = dctx.enter_context(tc.tile_pool(name="decw", bufs=1))
                dppool = dctx.enter_context(tc.tile_pool(name="decp", bufs=2, space="PSUM"))
                dmpool = dctx.enter_context(tc.tile_pool(name="decm", bufs=2, space="PSUM"))
                dspool = dctx.enter_context(tc.tile_pool(name="decs", bufs=2))
                d1pool = dctx.enter_context(tc.tile_pool(name="decs1", bufs=1))

                wih_d1 = wtiles2(WihT_dec1, KC, MC, 'wihd1', dwpool)
                whh_d0 = wtiles2(WhhT_dec[0], KC, MC, 'whhd0', dwpool)
                whh_d1 = wtiles2(WhhT_dec[1], KC, MC, 'whhd1', dwpool)
                wcx = wtiles2(WcxT, KC, KC, 'wcx', dwpool)
                wo = wtiles2(WoT_d, KC, 32, 'wo', dwpool)

                nc.vector.memset(btx[:], float(BOS))
                tok_init = dspool.tile([4, 1], f32, tag='tokinit')
                nc.sync.dma_start(tok_init[:], INIT_TOK[:])
                nc.vector.tensor_copy(btx[:, 64:65], tok_init[:])
                nc.sync.dma_start(scores[:], INIT_SC[:])

                ag_in = dpool.tile([128, 8], f32, tag='agin')
                tmp18 = dpool.tile([3, 6], f32, tag='tmp18')

                def lstm_cell_T(zs, coff):
                    """zs: SBUF [128, 64] pre-activations, gate m-order i,f,o,g."""
                    sio = dspool.tile([128, 48], f32, tag='dsio')
                    tg = dspool.tile([128, 16], f32, tag='dtg')
                    nc.scalar.activation(sio[:], zs[:, 0:48], AF.Sigmoid)
                    nc.scalar.activation(tg[:], zs[:, 48:64], AF.Tanh)
                    cs = hc[:, coff:coff + 16]
                    nc.vector.tensor_tensor(cs, sio[:, 16:32], cs, op=ALU.mult)
                    si2 = dspool.tile([128, 16], f32, tag='dsi2')
                    nc.vector.tensor_tensor(si2[:], sio[:, 0:16], tg[:], op=ALU.mult)
                    nc.vector.tensor_tensor(cs, cs, si2[:], op=ALU.add)
                    tcn = dspool.tile([128, 16], f32, tag='dtc')
                    nc.scalar.activation(tcn[:], cs, AF.Tanh)
                    hn = dspool.tile([128, 16], f32, tag='dhn')
                    nc.vector.tensor_tensor(hn[:], sio[:, 32:48], tcn[:], op=ALU.mult)
                    return hn

                NCAND = NC * 32 * 3   # 768

                # split of encoder-final h1/h2 for step-0 whh matmuls
                h1sp = split_tiles(dspool, hh[:, 0:16].bitcast(f32), 16, 'h1sp')
                h2sp = split_tiles(dspool, hh[:, 16:32].bitcast(f32), 16, 'h2sp')

                def gather_emb(tok_ap):
                    """offsets off[q] = MC*tok_{q%4} + q//4, gather EW rows."""
                    po = dppool.tile([MC * BS, 1], f32, tag='pmed')
                    nc.tensor.matmul(po[:], selb[:], tok_ap, start=True, stop=True)
                    offf = dspool.tile([MC * BS, 1], f32, tag='offf')
                    nc.vector.tensor_tensor(offf[:], po[:], moff[:], op=ALU.add)
                    offi = dspool.tile([MC * BS, 1], i32, tag='offi')
                    nc.vector.tensor_copy(offi[:], offf[:])
                    g2 = dspool.tile([MC * BS, 128], f32, tag='gEW')
                    nc.gpsimd.indirect_dma_start(
                        out=g2[:], out_offset=None, in_=EW4[:],
                        in_offset=bass.IndirectOffsetOnAxis(ap=offi[:, :1], axis=0))
                    return g2

                gEW = gather_emb(btx[:, 64:65])   # step-0 tokens (BOS)
                for step in range(T_steps):
                    first = (step == 0)
                    # --- layer 1: z1 = gEW.T (+ Whh0 @ h1T)
                    z1 = dppool.tile([128, 64], f32, tag='pbig')
                    if first:
                        nc.tensor.transpose(z1[:], gEW[:], eye[:64, :64])
                        for m in range(MC):
                            for k in range(KC):
                                mm3(z1[:, m * 4:(m + 1) * 4], whh_d0, MC, k, m,
                                    h1sp, (k * 4, (k + 1) * 4),
                                    start=False,
                                    stop=(m == MC - 1 and k == KC - 1))
                        z1s = dspool.tile([128, 64], f32, tag='z1s')
                        nc.vector.tensor_copy(z1s[:], z1[:])
                    else:
                        nc.tensor.transpose(z1[:], gEW[:], eye[:64, :64])
                        z1s = dspool.tile([128, 64], f32, tag='z1s')
                        nc.vector.tensor_tensor(z1s[:], z1[:], SM[:, 64:128], op=ALU.add)
                    h1n = lstm_cell_T(z1s, 16)
                    nc.vector.tensor_copy(hc[:, 0:16], h1n[:])
                    nc.vector.tensor_copy(hh[:, 0:16], h1n[:])
                    h1sp = split_tiles(dspool, h1n[:], 16, 'h1sp')
                    # --- layer 2: z2 = Wih1 @ h1new (+ Whh1 @ h2)
                    z2 = dppool.tile([128, 64], f32, tag='pbig')
                    for m in range(MC):
                        for k in range(KC):
                            mm3(z2[:, m * 4:(m + 1) * 4], wih_d1, MC, k, m,
                                h1sp, (k * 4, (k + 1) * 4),
                                start=(m == 0 and k == 0),
                                stop=(not first and m == MC - 1 and k == KC - 1))
                    if first:
                        for m in range(MC):
                            for k in range(KC):
                                mm3(z2[:, m * 4:(m + 1) * 4], whh_d1, MC, k, m,
                                    h2sp, (k * 4, (k + 1) * 4),
                                    start=False,
                                    stop=(m == MC - 1 and k == KC - 1))
                        z2s = dspool.tile([128, 64], f32, tag='z2s')
                        nc.vector.tensor_copy(z2s[:], z2[:])
                    else:
                        z2s = dspool.tile([128, 64], f32, tag='z2s')
                        nc.vector.tensor_tensor(z2s[:], z2[:], SM[:, 128:192], op=ALU.add)
                    h2n = lstm_cell_T(z2s, 48)
                    nc.vector.tensor_copy(hc[:, 32:48], h2n[:])
                    nc.vector.tensor_copy(hh[:, 16:32], h2n[:])
                    h2sp = split_tiles(dspool, h2n[:], 16, 'h2sp')
                    # --- attention
                    psc = dppool.tile([4, S], f32, tag='pmed')
                    for k in range(KC):
                        nc.tensor.matmul(
                            psc[:], hh[:, 16 + k * 4:16 + (k + 1) * 4],
                            enc_outT[:, k * S:(k + 1) * S],
                            start=(k == 0), stop=(k == KC - 1))
                    att_e = dspool.tile([4, S], f32, tag='atte')
                    den = dspool.tile([4, 1], f32, tag='den')
                    nc.scalar.activation(att_e[:], psc[:], AF.Exp, accum_out=den[:])
                    nc.vector.reciprocal(den[:], den[:])
                    attn = dspool.tile([4, S], f32, tag='attn')
                    nc.vector.tensor_scalar(attn[:], att_e[:], den[:], None, ALU.mult)
                    patT = dppool.tile([S, 4], f32, tag='pmed')
                    nc.tensor.transpose(patT[:], attn[:], eye[:4, :4])
                    attnT = dspool.tile([S, 4], f32r, tag='attnT')
                    nc.vector.tensor_copy(attnT[:], patT[:])
                    # --- featT = tanh(Wcx @ h2T + EOC.T @ attnT)
                    pft = dppool.tile([128, 16], f32, tag='pmed')
                    for m in range(KC):
                        for k in range(KC):
                            mm3(pft[:, m * 4:(m + 1) * 4], wcx, KC, k, m,
                                h2sp, (k * 4, (k + 1) * 4),
                                start=(m == 0 and k == 0), stop=False)
                    for m in range(KC):
                        nc.tensor.matmul(
                            pft[:, m * 4:(m + 1) * 4],
                            EOC[:, m * 128:(m + 1) * 128], attnT[:],
                            start=False, stop=(m == KC - 1))
                    feat = dspool.tile([128, 16], f32, tag='feat')
                    nc.scalar.activation(feat[:], pft[:], AF.Tanh)
                    fsp = split_tiles(dspool, feat[:], 16, 'fsp')
                    # --- logits [128, 128] cols m*4+b
                    pL = dppool.tile([128, 128], f32, tag='pbig')
                    for m in range(32):
                        for k in range(KC):
                            mm3(pL[:, m * 4:(m + 1) * 4], wo, 32, k, m,
                                fsp, (k * 4, (k + 1) * 4),
                                start=(m == 0 and k == 0),
                                stop=(m == 31 and k == KC - 1))
                    Ls = d1pool.tile([128, 128], f32, tag='Ls')
                    nc.vector.tensor_tensor(Ls[:], pL[:], lmask[:], op=ALU.add)
                    # --- per-core sumexp per beam
                    Ls3 = Ls[:].rearrange("p (m b) -> p m b", m=32, b=4)
                    sx = dspool.tile([128, 4], f32, tag='sx')
                    esc = d1pool.tile([128, 32], f32, tag='esc')
                    for b in range(3):
                        nc.scalar.activation(
                            esc[:].rearrange("p (m o) -> p m o", o=1),
                            Ls3[:, :, b:b + 1], AF.Exp,
                            accum_out=sx[:, b:b + 1])
                    nc.vector.memset(sx[:, 3:4], 0.0)
                    ps4 = dppool.tile([1, 4], f32, tag='pmed')
                    nc.tensor.matmul(ps4[:], onescol[:], sx[:], start=True, stop=True)
                    s4s = dspool.tile([1, 4], f32, tag='s4s')
                    nc.vector.tensor_copy(s4s[:], ps4[:])
                    ps4T = dppool.tile([4, 1], f32, tag='pmed')
                    nc.tensor.transpose(ps4T[:], s4s[:], eye[:1, :1])
                    # --- transpose logits; per-(chunk,beam) top-3 + ids
                    pLT = dppool.tile([128, 128], f32, tag='pbig')
                    nc.tensor.transpose(pLT[:], Ls[:], eye[:])
                    LT = d1pool.tile([128, 128], f32, tag='LT')
                    nc.vector.tensor_copy(LT[:], pLT[:])
                    mx = dspool.tile([128, 8], f32, tag='mx')
                    mi = dspool.tile([128, 8], u32, tag='mi')
                    nc.vector.max(mx[:], LT[:])
                    nc.vector.max_index(mi[:], mx[:], LT[:])
                    P = dspool.tile([128, 8], f32, tag='P')
                    nc.vector.tensor_copy(P[:, 0:3], mx[:, 0:3])
                    mif = dspool.tile([128, 3], f32, tag='mif')
                    nc.vector.tensor_copy(mif[:], mi[:, 0:3])
                    nc.vector.tensor_scalar(P[:, 3:6], mif[:], rowbase[:], None, ALU.add)
                    nc.vector.tensor_copy(P[0:4, 6:7], ps4T[:])
                    # --- AllGather
                    ag_out = dpool.tile([NC * 128, 8], f32, tag=f'agout{step}',
                                        addr_space="Shared")
                    nc.sync.dma_start(ag_in[:], P[:])
                    nc.gpsimd.collective_compute(
                        "AllGather", ALU.bypass,
                        replica_groups=[list(range(NC))],
                        ins=[ag_in[:].opt()], outs=[ag_out[:].opt()])
                    # --- M1/M2 precompute for next step (overlaps collective)
                    if step < T_steps - 1:
                        M1p = dmpool.tile([128, 64], f32, tag='pM')
                        for m in range(MC):
                            for k in range(KC):
                                mm3(M1p[:, m * 4:(m + 1) * 4], whh_d0, MC, k, m,
                                    h1sp, (k * 4, (k + 1) * 4),
                                    start=(m == 0 and k == 0),
                                    stop=(m == MC - 1 and k == KC - 1))
                        nc.vector.tensor_copy(SM[:, 64:128], M1p[:])
                        M2p = dmpool.tile([128, 64], f32, tag='pM')
                        for m in range(MC):
                            for k in range(KC):
                                mm3(M2p[:, m * 4:(m + 1) * 4], whh_d1, MC, k, m,
                                    h2sp, (k * 4, (k + 1) * 4),
                                    start=(m == 0 and k == 0),
                                    stop=(m == MC - 1 and k == KC - 1))
                        nc.vector.tensor_copy(SM[:, 128:192], M2p[:])
                    nc.vector.tensor_copy(SM[:, 0:64], hc[:])
                    cv = d1pool.tile([4, NCAND], f32, tag='cv')
                    nc.sync.dma_start(
                        cv[:].rearrange("b (c m r) -> b c m r", c=NC, m=32, r=3),
                        ag_out.rearrange("(c m b) r -> b c m r", c=NC, m=32, b=4)[:, :, :, 0:3])
                    ci = d1pool.tile([4, NCAND], f32, tag='ci')
                    nc.sync.dma_start(
                        ci[:].rearrange("b (c m r) -> b c m r", c=NC, m=32, r=3),
                        ag_out.rearrange("(c m b) r -> b c m r", c=NC, m=32, b=4)[:, :, :, 3:6])
                    ss = dspool.tile([4, NC], f32, tag='ss')
                    nc.sync.dma_start(
                        ss[:].rearrange("b (c o) -> b c o", o=1),
                        ag_out.rearrange("(c q) r -> q c r", c=NC, q=128)[0:4, :, 6:7])
                    # --- merge
                    ssum = dspool.tile([4, 1], f32, tag='ssum')
                    nc.vector.reduce_sum(ssum[:], ss[:], AX.X)
                    lse = dspool.tile([4, 1], f32, tag='lse')
                    nc.scalar.activation(lse[:], ssum[:], AF.Ln)
                    gv8 = dspool.tile([4, 8], f32, tag='gv8')
                    nc.vector.max(gv8[:], cv[:])
                    pos8 = dspool.tile([4, 8], u32, tag='pos8')
                    nc.vector.max_index(pos8[:], gv8[:], cv[:])
                    pos8f = dspool.tile([4, 8], f32, tag='pos8f')
                    nc.vector.tensor_copy(pos8f[:], pos8[:])
                    tokf9 = dspool.tile([4, 3], f32, tag='tokf9')
                    ohr = d1pool.tile([4, NCAND], f32, tag='ohr')
                    for r in range(3):
                        nc.vector.tensor_scalar(
                            ohr[:], iota768[:], pos8f[:, r:r + 1], None, ALU.is_equal)
                        nc.vector.tensor_tensor(ohr[:], ohr[:], ci[:], op=ALU.mult)
                        nc.vector.reduce_sum(tokf9[:, r:r + 1], ohr[:], AX.X)
                    # cum9 = gv8[:, :3] + (scores - lse)
                    sml = dspool.tile([4, 1], f32, tag='sml')
                    nc.vector.tensor_tensor(sml[:], scores[:], lse[:], op=ALU.subtract)
                    pack9 = dspool.tile([4, 8], f32, tag='pack9')
                    nc.vector.tensor_scalar(pack9[:, 0:3], gv8[:, 0:3], sml[:], None, ALU.add)
                    nc.vector.tensor_copy(pack9[:, 3:6], tokf9[:])
                    nc.sync.dma_start(tmp18[:], pack9[0:3, 0:6])
                    t3 = tmp18.rearrange("b (h j) -> b h j", h=2, j=3)
                    flat18 = dspool.tile([1, 18], f32, tag='flat18')
                    nc.sync.dma_start(
                        flat18[:, 0:9].rearrange("o (b j) -> o b j", b=3, j=3),
                        t3[:, 0:1, :].rearrange("b o j -> o b j"))
                    nc.sync.dma_start(
                        flat18[:, 9:18].rearrange("o (b j) -> o b j", b=3, j=3),
                        t3[:, 1:2, :].rearrange("b o j -> o b j"))
                    mx9 = dspool.tile([1, 8], f32, tag='mx9')
                    nc.vector.max(mx9[:], flat18[:, 0:9])
                    p9 = dspool.tile([1, 8], u32, tag='p9')
                    nc.vector.max_index(p9[:], mx9[:], flat18[:, 0:9])
                    p9f = dspool.tile([1, 8], f32, tag='p9f')
                    nc.vector.tensor_copy(p9f[:], p9[:])
                    pp9 = dppool.tile([4, 1], f32, tag='pmed')
                    nc.tensor.transpose(pp9[:], p9f[:, 0:4], eye[:1, :1])
                    p9P = dspool.tile([4, 1], f32, tag='p9P')
                    nc.vector.tensor_copy(p9P[:], pp9[:])
                    OH = dspool.tile([4, 9], f32, tag='OH')
                    nc.vector.tensor_scalar(OH[:], iota9[:], p9P[:], None, ALU.is_equal)
                    pRep = dppool.tile([4, 18], f32, tag='pmed')
                    nc.tensor.matmul(pRep[:], ones1[:, 0:4], flat18[:],
                                     start=True, stop=True)
                    tmp9 = dspool.tile([4, 9], f32, tag='tmp9')
                    tokN = dspool.tile([4, 1], f32, tag='tokN')
                    nc.vector.tensor_tensor(tmp9[:], OH[:], pRep[:, 9:18], op=ALU.mult)
                    nc.vector.reduce_sum(tokN[:], tmp9[:], AX.X)
                    scN = dspool.tile([4, 1], f32, tag='scN')
                    nc.vector.tensor_tensor(tmp9[:], OH[:], pRep[:, 0:9], op=ALU.mult)
                    nc.vector.reduce_sum(scN[:], tmp9[:], AX.X)
                    origN = dspool.tile([4, 1], f32, tag='origN')
                    nc.vector.tensor_tensor(tmp9[:], OH[:], orig9[:], op=ALU.mult)
                    nc.vector.reduce_sum(origN[:], tmp9[:], AX.X)
                    # --- prefetch next step's embedding rows (tokens known now)
                    if step < T_steps - 1:
                        gEW = gather_emb(tokN[:])
                    # --- selection matrices
                    porow = dppool.tile([1, 4], f32, tag='pmed')
                    nc.tensor.transpose(porow[:], origN[:], eye[:4, :4])
                    orow = dspool.tile([1, 4], f32, tag='orow')
                    nc.vector.tensor_copy(orow[:], porow[:])
                    pOR = dppool.tile([128, 4], f32, tag='pmed')
                    nc.tensor.matmul(pOR[:], ones1[:], orow[:], start=True, stop=True)
                    ORs = dspool.tile([128, 4], f32, tag='ORs')
                    nc.vector.tensor_copy(ORs[:], pOR[:])
                    SelRep = dspool.tile([128, 16], f32, tag='SelRep')
                    nc.vector.tensor_tensor(
                        SelRep[:].rearrange("p (a b) -> p a b", a=4, b=4),
                        ORs[:].rearrange("p (o b) -> p o b", o=1, b=4)
                        .to_broadcast([128, 4, 4]),
                        ac16[:].rearrange("p (a b) -> p a b", a=4, b=4),
                        op=ALU.is_equal)
                    SelT = dspool.tile([4, 4], f32, tag='SelT')
                    nc.vector.tensor_scalar(SelT[:], ORs[0:4, :], iotap4[:],
                                            None, ALU.is_equal)
                    # --- reorder SM (hc first to unblock hh, then M1|M2)
                    smn = d1pool.tile([128, 192], f32, tag='smn')
                    tmpa = d1pool.tile([128, 192], f32, tag='tmpa')
                    srv = SelRep[:].rearrange("p (a b) -> p a b", a=4, b=4)
                    for lo, gk in ((0, 16), (64, 32)):
                        smv = SM[:, lo:lo + gk * 4].rearrange(
                            "p (gk b) -> p gk b", gk=gk, b=4)
                        for a in range(4):
                            dst = smn if a == 0 else tmpa
                            nc.vector.tensor_tensor(
                                dst[:, lo:lo + gk * 4].rearrange(
                                    "p (gk b) -> p gk b", gk=gk, b=4),
                                smv[:, :, a:a + 1].to_broadcast([128, gk, 4]),
                                srv[:, a:a + 1, :].to_broadcast([128, gk, 4]),
                                op=ALU.mult)
                            if a > 0:
                                nc.vector.tensor_tensor(
                                    smn[:, lo:lo + gk * 4], smn[:, lo:lo + gk * 4],
                                    tmpa[:, lo:lo + gk * 4], op=ALU.add)
                        nc.vector.tensor_copy(SM[:, lo:lo + gk * 4],
                                              smn[:, lo:lo + gk * 4])
                        if lo == 0:
                            nc.vector.tensor_copy(hc[:], SM[:, 0:64])
                            nc.vector.tensor_copy(hh[:, 0:16], hc[:, 0:16])
                            nc.vector.tensor_copy(hh[:, 16:32], hc[:, 32:48])
                    # --- backtrack + new tokens/scores
                    pbt = dppool.tile([4, 72], f32, tag='pmed')
                    nc.tensor.matmul(pbt[:, 0:65], SelT[:], btx[:, 0:65],
                                     start=True, stop=True)
                    if not first:
                        nc.vector.tensor_copy(btx[:, 0:64], pbt[:, 0:64])
                        nc.vector.tensor_copy(btx[:, step:step + 1], pbt[:, 64:65])
                    nc.vector.tensor_copy(btx[:, 64:65], tokN[:])
                    nc.vector.tensor_copy(scores[:], scN[:])

                # --- final: pick best beam
                srow = dspool.tile([1, 8], f32, tag='srow')
                nc.vector.tensor_copy(srow[:], neg8[:])
                psr = dppool.tile([1, 4], f32, tag='pmed')
                nc.tensor.transpose(psr[:], scores[:], eye[:4, :4])
                nc.vector.tensor_copy(srow[:, 0:4], psr[:])
                fmx = dspool.tile([1, 8], f32, tag='fmx')
                nc.vector.max(fmx[:], srow[:])
                fmi = dspool.tile([1, 8], u32, tag='fmi')
                nc.vector.max_index(fmi[:], fmx[:], srow[:])
                wf = dspool.tile([1, 1], f32, tag='wf')
                nc.vector.tensor_copy(wf[:], fmi[:, 0:1])
                pwr = dppool.tile([4, 1], f32, tag='pmed')
                nc.tensor.matmul(pwr[:], ones1[:, 0:4], wf[:], start=True, stop=True)
                wrs = dspool.tile([4, 1], f32, tag='wrs')
                nc.vector.tensor_copy(wrs[:], pwr[:])
                oh4 = dspool.tile([4, 1], f32, tag='oh4')
                nc.vector.tensor_tensor(oh4[:], wrs[:], iotap4[:], op=ALU.is_equal)
                pout = dppool.tile([1, 64], f32, tag='pmed')
                nc.tensor.matmul(pout[:], oh4[:], btx[:, 0:64], start=True, stop=True)
                outi = dspool.tile([1, 64], i32, tag='outi')
                nc.vector.tensor_copy(outi[:], pout[:])
                nc.sync.dma_start(OUT[:], outi[:, 0:TMAX])

    nc.compile()
    return nc


def kernel(**inputs):
    com, per_core = host_prep(inputs)
    nc = build(T_steps=TMAX)
    in_maps = []
    for c in range(NC):
        m = dict(com)
        m.update(per_core[c])
        in_maps.append(m)
    res = bass_utils.run_bass_kernel_spmd(nc, in_maps, core_ids=list(range(NC)))
    out = res.results[0]['out_tokens'][0]
    return out.astype(np.int32)


if __name__ == '__main__':
    import os
    here = os.path.dirname(os.path.abspath(__file__))
    inputs = dict(np.load(os.path.join(here, 'work', 'inputs.npz')))
    out = kernel(**inputs)
    print('kernel out:', out[:20])

